# revision 12
# baseline (speedup 1.0000x reference)
"""Trainium2 Bass kernel for nn_Attention_40716289966507.

Reference computation (B=4, C=256, H=W=48, heads=8, d=32, N=H*W=2304):
    qkv = w_qkv @ x            # 1x1 conv -> q,k,v each [B, 256, N]
    attn = softmax(q^T k / sqrt(d))   per (batch, head): [N, N]
    out  = v @ attn^T          # [B, 256, N]
    y    = w_proj @ out + b    # [B, 256, N]

Sharding (8 cores): core i handles batch b = i//2 and query-token half
t = i%2 (1152 of the 2304 tokens). Each core needs the full image of its
batch (for K and V) but only its token half for Q; it produces the full
256-channel output for its 1152 tokens, so the host just concatenates —
no cross-core reduction.

The v1 kernel was ACT(exp)-bound: 21.2M softmax exps per core at the
scalar engine's 1 elem/cycle/lane = ~138us floor. v2 splits the exp
work across THREE engines, assigned per pair-tile by a weighted deficit
scheduler:
  * ACT: exact exp (scale folded into the activation affine) -> fp16.
  * DVE / GPSIMD: Schraudolph-style fp16 "magic" exp in ONE
    tensor_scalar: u = trunc(s * (1024*log2e*SCALE) + c1) as int16,
    whose bit pattern IS the fp16 approximation of exp(s*SCALE)
    (piecewise-linear-in-mantissa, log-err std ~1.8%, sigma-centered).
    The int16 result is written through a bitcast view directly into
    the fp16 pt tile the AV matmuls consume. Softmax renormalization
    cancels the row-mean error; measured end-to-end rel-err with this
    split is ~4e-3 vs the 2e-2 gate.
Per-core pipeline otherwise as v1: qkv matmuls f32r; q/k copied to
fp16 (kills the f32r short-tile 4x penalty in the tail S^T matmuls); v
materialized transposed in fp16 off the tensor engine; attention per
4-head group / query tile / 128-key chunk with row-packed S^T (K=32),
one exp per 2-head pair tile, col-packed AV + ones-matmul softmax
denominators accumulating in PSUM; per-channel-normalized by DVE;
proj f32r + bias; per-chunk output DMA. PSUM->SBUF qkv/v copies run on
GPSIMD so DVE keeps capacity for exp.
"""

import numpy as np

import concourse.bacc as bacc
import concourse.mybir as mybir
import concourse.tile as tile

F32 = mybir.dt.float32
F32R = mybir.dt.float32r
FP16 = mybir.dt.float16
I16 = mybir.dt.int16

P = 128
C = 256          # channels
N = 2304         # tokens per image
NQ = 1152        # query tokens per core
D = 32           # head dim
KC = N // P      # 18 key chunks
SCALE = D ** -0.5
QT = [(0, 512), (512, 512)]   # full query tiles; 1024:1152 tail is a merged pass
NT3 = 384        # free-dim tile for qkv/proj matmuls (1152 = 3*384)

# magic-exp constants: u = trunc(s*MAGIC_C0 + MAGIC_C1) -> int16 bits of
# fp16 ~= exp(s*SCALE) * 2^(gamma - sigma); sigma centers the log error.
_L2E = float(np.log2(np.e))
_SIGMA = 0.0397 / float(np.log(2.0))     # mean of ln((1+t)/2^t) in log2 units
MAGIC_C0 = 1024.0 * _L2E * SCALE
MAGIC_C1 = 1024.0 * (15.0 - _SIGMA) + 0.5

# exp work shares (elements) per engine: ACT exact / DVE magic.
# (GPSIMD cannot access PSUM, and every elementwise op here reads PSUM, so
# only ACT and DVE can share the work. ACT also does the qkv PSUM->SBUF
# copies and the proj bias-add; DVE does reciprocal + av normalize.)
EXP_SHARES = {"A": 0.486, "D": 0.514}


class ExpSplit:
    """Weighted deficit round-robin: assign each exp tile to the engine
    with the largest remaining share (stride scheduling)."""

    def __init__(self, shares):
        self.shares = dict(shares)
        self.done = {e: 0.0 for e in shares}

    def pick(self, els):
        e = min(self.done, key=lambda k: (self.done[k] + els) / self.shares[k])
        self.done[e] += els
        return e


def emit(tc, iters=1):
    from contextlib import ExitStack
    ctx = ExitStack()
    nc = tc.nc
    xq_d = nc.dram_tensor("xq", [C, NQ], F32R, kind="ExternalInput").ap()
    xf_d = nc.dram_tensor("xf", [C, N], F32R, kind="ExternalInput").ap()
    wqkvT_d = nc.dram_tensor("wqkvT", [C, 3 * C], F32R, kind="ExternalInput").ap()
    wprojT_d = nc.dram_tensor("wprojT", [C, C], F32R, kind="ExternalInput").ap()
    bprojT_d = nc.dram_tensor("bprojT", [P, 2], F32, kind="ExternalInput").ap()
    y_d = nc.dram_tensor("y", [C, NQ], F32, kind="ExternalOutput").ap()

    singles = ctx.enter_context(tc.tile_pool(name="singles", bufs=1))
    acts = ctx.enter_context(tc.tile_pool(name="acts", bufs=1))
    qkv_ps = ctx.enter_context(tc.tile_pool(name="qkv_ps", bufs=2, space="PSUM"))
    st_ps = ctx.enter_context(tc.tile_pool(name="st_ps", bufs=2, space="PSUM"))
    av_ps = ctx.enter_context(tc.tile_pool(name="av_ps", bufs=1, space="PSUM"))
    sm_ps = ctx.enter_context(tc.tile_pool(name="sm_ps", bufs=1, space="PSUM"))
    pt_pool = ctx.enter_context(tc.tile_pool(name="pt", bufs=4))
    small = ctx.enter_context(tc.tile_pool(name="small", bufs=2))

    split = ExpSplit(EXP_SHARES)

    # preload the exp table while DMAs/qkv run
    warm = singles.tile([P, 8], F32)
    nc.vector.memset(warm[:], 0.0)
    warm2 = singles.tile([P, 8], F32)
    nc.scalar.activation(warm2[:], warm[:], mybir.ActivationFunctionType.Exp)

    ones_sb = singles.tile([P, D], FP16)
    nc.vector.memset(ones_sb[:], 1.0)
    bias_sb = singles.tile([P, 2], F32)
    nc.sync.dma_start(bias_sb[:], bprojT_d)

    # weights: per-ki-chunk DMAs for early starts
    wq_sb = singles.tile([P, 2, 3 * C], F32R)
    wqkvT_r = wqkvT_d.rearrange("(ki p) o -> p ki o", p=P)
    for sec in range(3):          # q, k, v weight sections separately so
        for ki in range(2):       # the q matmuls start after ~1/3 the bytes
            sl = slice(sec * C, (sec + 1) * C)
            nc.sync.dma_start(wq_sb[:, ki, sl], wqkvT_r[:, ki, sl])
    wp_sb = singles.tile([P, 2, C], F32R)
    nc.sync.dma_start(wp_sb[:], wprojT_d.rearrange("(ki p) o -> p ki o", p=P))

    # x: query half and full image, split by (ki, token range)
    xq_sb = singles.tile([P, 2, NQ], F32R)
    xq_r = xq_d.rearrange("(ki p) n -> p ki n", p=P)
    for ki in range(2):
        for nt in range(NQ // NT3):
            sl = slice(nt * NT3, (nt + 1) * NT3)
            nc.sync.dma_start(xq_sb[:, ki, sl], xq_r[:, ki, sl])
    xf_sb = singles.tile([P, 2, N], F32R)
    xf_r = xf_d.rearrange("(ki p) n -> p ki n", p=P)
    for ki in range(2):
        for nt in range(N // NT3):
            sl = slice(nt * NT3, (nt + 1) * NT3)
            nc.sync.dma_start(xf_sb[:, ki, sl], xf_r[:, ki, sl])

    # per-group activations (separate tiles => fine-grained deps)
    q_g = [acts.tile([P, NQ], FP16, name=f"q{g}") for g in range(2)]
    k_g = [acts.tile([P, N], FP16, name=f"k{g}") for g in range(2)]
    vT_c = [acts.tile([P, C], FP16, name=f"vt{mo}") for mo in range(KC)]
    av_sb = acts.tile([P, 2, NQ], F32R)
    y_sb = acts.tile([P, 2, NQ], F32)

    mm = nc.tensor.matmul

    def qkv_mm(dst_tile, w_col0, rhs_sb, nt):
        sl = slice(nt * NT3, (nt + 1) * NT3)
        ps = qkv_ps.tile([P, NT3], F32, tag="qkv")
        for ki in range(2):
            mm(ps[:], wq_sb[:, ki, w_col0:w_col0 + P], rhs_sb[:, ki, sl],
               start=(ki == 0), stop=(ki == 1))
        nc.scalar.copy(dst_tile[:, sl], ps[:])

    def emit_qkv_group(g):
        # q rows for group g = channels 128g..128g+127; k = 256+128g..
        for nt in range(NQ // NT3):
            qkv_mm(q_g[g], g * P, xq_sb, nt)
        for nt in range(N // NT3):
            qkv_mm(k_g[g], C + g * P, xf_sb, nt)

    def emit_vt(mo):
        ps = qkv_ps.tile([P, NT3], F32, tag="qkv")
        for ki in range(2):
            mm(ps[:, :C], xf_sb[:, ki, mo * P:(mo + 1) * P],
               wq_sb[:, ki, 2 * C:3 * C],
               start=(ki == 0), stop=(ki == 1))
        nc.scalar.copy(vT_c[mo][:], ps[:, :C])

    def emit_exp(pt_dst, st_src, els):
        eng = split.pick(els)
        if eng == "A":
            nc.scalar.activation(pt_dst, st_src,
                                 mybir.ActivationFunctionType.Exp,
                                 scale=SCALE)
        else:
            nc.vector.tensor_scalar(pt_dst.bitcast(I16), st_src,
                                    MAGIC_C0, MAGIC_C1,
                                    mybir.AluOpType.mult, mybir.AluOpType.add)

    def emit_attention(g):
        for (q0, qtw) in QT:
            av = av_ps.tile([P, 512], F32)
            sm = sm_ps.tile([P, 512], F32)
            for kc in range(KC):
                # two 2-head pair tiles, pool bufs=2: the PE fills one
                # pair's banks while the exp engine still reads the other
                # pair of the previous chunk -> no exp->S^T serialization.
                pt = pt_pool.tile([P, 4, 512], FP16)
                for pair in range(2):
                    st = st_ps.tile([P, 2, 512], F32, tag="st")
                    for hh in range(2):
                        h = 2 * pair + hh
                        mm(st[:, hh, :qtw],
                           k_g[g][32 * h:32 * (h + 1), kc * P:(kc + 1) * P],
                           q_g[g][32 * h:32 * (h + 1), q0:q0 + qtw],
                           start=True, stop=True,
                           tile_position=(32 * h, 0))
                    emit_exp(pt[:, 2 * pair:2 * pair + 2, :qtw],
                             st[:, :, :qtw], 2 * qtw)
                for h in range(4):
                    mm(av[32 * h:32 * (h + 1), :qtw],
                       vT_c[kc][:, 128 * g + 32 * h:128 * g + 32 * (h + 1)],
                       pt[:, h, :qtw],
                       start=(kc == 0), stop=(kc == KC - 1),
                       tile_position=(0, 32 * h), skip_group_check=True)
                for h in range(4):
                    mm(sm[32 * h:32 * (h + 1), :qtw],
                       ones_sb[:, :],
                       pt[:, h, :qtw],
                       start=(kc == 0), stop=(kc == KC - 1),
                       tile_position=(0, 32 * h), skip_group_check=True)
            rec = small.tile([P, 512], F32, tag="rec")
            nc.vector.reciprocal(rec[:, :qtw], sm[:, :qtw])
            nc.vector.tensor_mul(av_sb[:, g, q0:q0 + qtw], av[:, :qtw],
                                 rec[:, :qtw])

    def emit_tail():
        # queries 1024:1152 for BOTH groups in one pass: head bank h holds
        # g0 at cols 0:128, g1 at cols 128:256. Same-row-group matmuls into
        # one bank serialize on the PE (same cells), so no concurrent
        # same-bank drains.
        q0, qtw = 1024, 128
        av = av_ps.tile([P, 512], F32)
        sm = sm_ps.tile([P, 512], F32)
        for kc in range(KC):
            pt = pt_pool.tile([P, 4, 512], FP16)
            for pair in range(2):
                st = st_ps.tile([P, 2, 512], F32, tag="st")
                for g in range(2):
                    for hh in range(2):
                        h = 2 * pair + hh
                        mm(st[:, hh, g * qtw:(g + 1) * qtw],
                           k_g[g][32 * h:32 * (h + 1), kc * P:(kc + 1) * P],
                           q_g[g][32 * h:32 * (h + 1), q0:q0 + qtw],
                           start=(g == 0), stop=(g == 1),
                           tile_position=(32 * h, 0), skip_group_check=True)
                emit_exp(pt[:, 2 * pair:2 * pair + 2, :2 * qtw],
                         st[:, :, :2 * qtw], 4 * qtw)
            for g in range(2):
                for h in range(4):
                    mm(av[32 * h:32 * (h + 1), g * qtw:(g + 1) * qtw],
                       vT_c[kc][:, 128 * g + 32 * h:128 * g + 32 * (h + 1)],
                       pt[:, h, g * qtw:(g + 1) * qtw],
                       start=(kc == 0 and g == 0), stop=(kc == KC - 1 and g == 1),
                       tile_position=(0, 32 * h), skip_group_check=True)
            for g in range(2):
                for h in range(4):
                    mm(sm[32 * h:32 * (h + 1), g * qtw:(g + 1) * qtw],
                       ones_sb[:, :],
                       pt[:, h, g * qtw:(g + 1) * qtw],
                       start=(kc == 0 and g == 0), stop=(kc == KC - 1 and g == 1),
                       tile_position=(0, 32 * h), skip_group_check=True)
        rec = small.tile([P, 512], F32, tag="rec")
        nc.vector.reciprocal(rec[:, :2 * qtw], sm[:, :2 * qtw])
        for g in range(2):
            nc.vector.tensor_mul(av_sb[:, g, q0:q0 + qtw],
                                 av[:, g * qtw:(g + 1) * qtw],
                                 rec[:, g * qtw:(g + 1) * qtw])

    y_r = y_d.rearrange("(co p) n -> p co n", p=P)

    def emit_proj(co, nt):
        sl = slice(nt * NT3, (nt + 1) * NT3)
        ps = qkv_ps.tile([P, NT3], F32, tag="qkv")
        for ki in range(2):
            mm(ps[:], wp_sb[:, ki, co * P:(co + 1) * P],
               av_sb[:, ki, sl],
               start=(ki == 0), stop=(ki == 1))
        nc.scalar.activation(y_sb[:, co, sl], ps[:],
                             mybir.ActivationFunctionType.Identity,
                             bias=bias_sb[:, co:co + 1])
        nc.sync.dma_start(y_r[:, co, sl], y_sb[:, co, sl])

    # emission order shapes Tile's priorities: group A's inputs first so
    # the first exp lands as early as possible. proj chunks for queries
    # 0:768 only need av_sb written by the full query tiles, so they are
    # emitted before the merged tail pass and fill its engine gaps; the
    # last proj chunk (queries 768:1152) follows the tail. Output DMA is
    # per-chunk so results ship while later chunks still compute.
    # iters>1 repeats the body (same tiles; Tile serializes cross-iteration
    # hazards) — used only for steady-state timing measurements.
    for _ in range(iters):
        emit_qkv_group(0)
        for mo in range(KC):
            emit_vt(mo)
        emit_attention(0)
        emit_qkv_group(1)
        emit_attention(1)
        for co in range(2):
            for nt in range(2):
                emit_proj(co, nt)
        emit_tail()
        for co in range(2):
            emit_proj(co, 2)
    ctx.close()


_NC_CACHE = None


def build_nc():
    global _NC_CACHE
    if _NC_CACHE is None:
        nc = bacc.Bacc("TRN2", target_bir_lowering=False, debug=False,
                       num_devices=8)
        with tile.TileContext(nc) as tc:
            emit(tc)
        nc.compile()
        _NC_CACHE = nc
    return _NC_CACHE


def make_in_maps(x, w_qkv, w_proj, b_proj):
    x = np.ascontiguousarray(np.asarray(x, np.float32)).reshape(4, C, N)
    wqkvT = np.ascontiguousarray(np.asarray(w_qkv, np.float32).T)
    wprojT = np.ascontiguousarray(np.asarray(w_proj, np.float32).T)
    bprojT = np.ascontiguousarray(np.asarray(b_proj, np.float32).reshape(2, P).T)
    in_maps = []
    for core in range(8):
        b, t = divmod(core, 2)
        in_maps.append({
            "xq": np.ascontiguousarray(x[b][:, t * NQ:(t + 1) * NQ]),
            "xf": x[b],
            "wqkvT": wqkvT,
            "wprojT": wprojT,
            "bprojT": bprojT,
        })
    return in_maps


def assemble_output(results):
    y = np.empty((4, C, N), np.float32)
    for core in range(8):
        b, t = divmod(core, 2)
        y[b][:, t * NQ:(t + 1) * NQ] = results[core]["y"]
    return y.reshape(4, C, 48, 48)


def kernel(x, w_qkv, w_proj, b_proj):
    from concourse.bass_utils import run_bass_kernel_spmd
    nc = build_nc()
    in_maps = make_in_maps(x, w_qkv, w_proj, b_proj)
    res = run_bass_kernel_spmd(nc, in_maps, core_ids=list(range(8)))
    return assemble_output(res.results)


# revision 35
# speedup vs baseline: 2.2528x; 2.2528x over previous
"""Trainium2 Bass kernel for nn_Attention_40716289966507.

Reference computation (B=4, C=256, H=W=48, heads=8, d=32, N=H*W=2304):
    qkv = w_qkv @ x            # 1x1 conv -> q,k,v each [B, 256, N]
    attn = softmax(q^T k / sqrt(d))   per (batch, head): [N, N]
    out  = v @ attn^T          # [B, 256, N]
    y    = w_proj @ out + b    # [B, 256, N]

Sharding (8 cores): core i handles batch b = i//2 and query-token half
t = i%2 (1152 of the 2304 tokens). Each core needs the full image of its
batch (for K and V) but only its token half for Q; it produces the full
256-channel output for its 1152 tokens, so the host just concatenates —
no cross-core reduction.

The v1 kernel was ACT(exp)-bound: 21.2M softmax exps per core at the
scalar engine's 1 elem/cycle/lane = ~138us floor. This version splits
the elementwise work across BOTH PSUM-capable engines (GPSIMD cannot
access PSUM), assigned per tile by a weighted deficit scheduler:
  * ACT: exact exp (softmax scale folded into the activation affine)
    -> fp16 pt tile.
  * DVE: Schraudolph-style fp16 "magic" exp in ONE tensor_scalar:
    u = trunc(s * (1024*log2e*SCALE) + c1) as int16, whose bit pattern
    IS the fp16 approximation of exp(s*SCALE) (piecewise-linear in the
    mantissa, log-err std ~1.8%, sigma-centered). Written through a
    bitcast view directly into the fp16 pt tile the AV matmuls read.
    Softmax renormalization cancels the row-mean error; measured
    end-to-end rel-err is ~5e-3 vs the 2e-2 gate.
  * The PSUM->SBUF q/k/v copies also float between ACT and DVE via the
    same deficit scheduler; reciprocal/normalize stay on DVE
    (reciprocal_approx_fast, single custom-DVE instruction), the proj
    bias-add on ACT. (Keeping qkv_ps double-buffered matters: bufs=1
    measurably stalls the group-B qkv overlap under attention-A.)
Pipeline: qkv matmuls f32r; q/k copied to fp16 (kills the f32r
short-tile 4x penalty in the tail S^T matmuls); v materialized
transposed in fp16 off the tensor engine; attention per 4-head group /
query tile / 128-key chunk: all 4 row-packed S^T matmuls (K=32 at row
groups 32h) are emitted adjacently so the PE streams them concurrently
(512 cycles for 4 heads), then the two 2-head exps run on ACT and DVE
in parallel; col-packed AV + ones-matmul softmax denominators
accumulate in PSUM (col-packed matmuls stream concurrently on TRN2 —
microbenchmarked 213ns per 4x[128x32x512] group); proj f32r + bias;
per-chunk output DMA.
"""

import numpy as np

import concourse.bacc as bacc
import concourse.mybir as mybir
import concourse.tile as tile

F32 = mybir.dt.float32
F32R = mybir.dt.float32r
FP16 = mybir.dt.float16
I16 = mybir.dt.int16

P = 128
C = 256          # channels
N = 2304         # tokens per image
NQ = 1152        # query tokens per core
D = 32           # head dim
KC = N // P      # 18 key chunks
SCALE = D ** -0.5
QT = [(0, 512), (512, 512)]   # full query tiles; 1024:1152 tail is a merged pass
NT3 = 384        # free-dim tile for qkv/proj matmuls (1152 = 3*384)

# magic-exp constants: u = trunc(s*MAGIC_C0 + MAGIC_C1) -> int16 bits of
# fp16 ~= exp(s*SCALE) * 2^(gamma - sigma); sigma centers the log error.
_L2E = float(np.log2(np.e))
_SIGMA = 0.0397 / float(np.log(2.0))     # mean of ln((1+t)/2^t) in log2 units
MAGIC_C0 = 1024.0 * _L2E * SCALE
MAGIC_C1 = 1024.0 * (15.0 - _SIGMA) + 0.5

# exp work shares (elements) per engine: ACT exact / DVE magic.
# (GPSIMD cannot access PSUM, and every elementwise op here reads PSUM, so
# only ACT and DVE can share the work. ACT also does the qkv PSUM->SBUF
# copies and the proj bias-add; DVE does reciprocal + av normalize.)
EXP_SHARES = {"A": 0.50, "D": 0.50}


class ExpSplit:
    """Weighted deficit round-robin: assign each exp tile to the engine
    with the largest remaining share (stride scheduling). Calling pick()
    once per pair-tile makes the two pairs of a chunk usually land on
    different engines (so both engines work on the chunk concurrently),
    while the deficit logic still converges to the target shares."""

    def __init__(self, shares):
        self.shares = dict(shares)
        self.done = {e: 0.0 for e in shares}

    def pick(self, els):
        e = min(self.done, key=lambda k: (self.done[k] + els) / self.shares[k])
        self.done[e] += els
        return e


def emit(tc, iters=1, db=False):
    from contextlib import ExitStack
    ctx = ExitStack()
    nc = tc.nc
    xq_d = nc.dram_tensor("xq", [C, NQ], F32R, kind="ExternalInput").ap()
    xf_d = nc.dram_tensor("xf", [C, N], F32R, kind="ExternalInput").ap()
    wqkvT_d = nc.dram_tensor("wqkvT", [C, 3 * C], F32R, kind="ExternalInput").ap()
    wprojT_d = nc.dram_tensor("wprojT", [C, C], F32R, kind="ExternalInput").ap()
    bprojT_d = nc.dram_tensor("bprojT", [P, 2], F32, kind="ExternalInput").ap()
    y_d = nc.dram_tensor("y", [C, NQ], F32, kind="ExternalOutput").ap()

    singles = ctx.enter_context(tc.tile_pool(name="singles", bufs=1))
    acts = ctx.enter_context(tc.tile_pool(name="acts", bufs=2 if db else 1))
    qkv_ps = ctx.enter_context(tc.tile_pool(name="qkv_ps", bufs=2, space="PSUM"))
    st_ps = ctx.enter_context(tc.tile_pool(name="st_ps", bufs=2, space="PSUM"))
    av_ps = ctx.enter_context(tc.tile_pool(name="av_ps", bufs=1, space="PSUM"))
    sm_ps = ctx.enter_context(tc.tile_pool(name="sm_ps", bufs=1, space="PSUM"))
    pt_pool = ctx.enter_context(tc.tile_pool(name="pt", bufs=6))
    small = ctx.enter_context(tc.tile_pool(name="small", bufs=2))

    split = ExpSplit(EXP_SHARES)

    # preload the exp table while DMAs/qkv run
    warm = singles.tile([P, 8], F32)
    nc.vector.memset(warm[:], 0.0)
    warm2 = singles.tile([P, 8], F32)
    nc.scalar.activation(warm2[:], warm[:], mybir.ActivationFunctionType.Exp)

    ones_sb = singles.tile([P, D], FP16)
    nc.vector.memset(ones_sb[:], 1.0)
    bias_sb = singles.tile([P, 2], F32)
    nc.sync.dma_start(bias_sb[:], bprojT_d)

    # weights: per-ki-chunk DMAs for early starts
    wq_sb = singles.tile([P, 2, 3 * C], F32R)
    wqkvT_r = wqkvT_d.rearrange("(ki p) o -> p ki o", p=P)
    for sec in range(3):          # q, k, v weight sections separately so
        for ki in range(2):       # the q matmuls start after ~1/3 the bytes
            sl = slice(sec * C, (sec + 1) * C)
            nc.sync.dma_start(wq_sb[:, ki, sl], wqkvT_r[:, ki, sl])
    wp_sb = singles.tile([P, 2, C], F32R)
    nc.sync.dma_start(wp_sb[:], wprojT_d.rearrange("(ki p) o -> p ki o", p=P))

    # x: query half and full image, split by (ki, token range)
    xq_sb = singles.tile([P, 2, NQ], F32R)
    xq_r = xq_d.rearrange("(ki p) n -> p ki n", p=P)
    for ki in range(2):
        for nt in range(NQ // NT3):
            sl = slice(nt * NT3, (nt + 1) * NT3)
            nc.sync.dma_start(xq_sb[:, ki, sl], xq_r[:, ki, sl])
    xf_sb = singles.tile([P, 2, N], F32R)
    xf_r = xf_d.rearrange("(ki p) n -> p ki n", p=P)
    for ki in range(2):
        for nt in range(N // NT3):
            sl = slice(nt * NT3, (nt + 1) * NT3)
            nc.sync.dma_start(xf_sb[:, ki, sl], xf_r[:, ki, sl])

    mm = nc.tensor.matmul

    def qkv_mm(dst_tile, w_col0, rhs_sb, nt):
        sl = slice(nt * NT3, (nt + 1) * NT3)
        ps = qkv_ps.tile([P, NT3], F32, tag="qkv")
        for ki in range(2):
            mm(ps[:], wq_sb[:, ki, w_col0:w_col0 + P], rhs_sb[:, ki, sl],
               start=(ki == 0), stop=(ki == 1))
        emit_copy(dst_tile[:, sl], ps[:], NT3)

    def emit_qkv_group(g):
        # q rows for group g = channels 128g..128g+127; k = 256+128g..
        for nt in range(NQ // NT3):
            qkv_mm(q_g[g], g * P, xq_sb, nt)
        for nt in range(N // NT3):
            qkv_mm(k_g[g], C + g * P, xf_sb, nt)

    def emit_vt(mo):
        ps = qkv_ps.tile([P, NT3], F32, tag="qkv")
        for ki in range(2):
            mm(ps[:, :C], xf_sb[:, ki, mo * P:(mo + 1) * P],
               wq_sb[:, ki, 2 * C:3 * C],
               start=(ki == 0), stop=(ki == 1))
        emit_copy(vT_c[mo][:], ps[:, :C], C)

    def emit_copy(dst, src, els):
        # PSUM->SBUF copies can run on either exp engine; let the deficit
        # scheduler absorb them into whichever is ahead.
        if split.pick(els) == "A":
            nc.scalar.copy(dst, src)
        else:
            nc.vector.tensor_copy(dst, src)

    def emit_exp(pt_dst, st_src, els):
        eng = split.pick(els)
        if eng == "A":
            nc.scalar.activation(pt_dst, st_src,
                                 mybir.ActivationFunctionType.Exp,
                                 scale=SCALE)
        else:
            nc.vector.tensor_scalar(pt_dst.bitcast(I16), st_src,
                                    MAGIC_C0, MAGIC_C1,
                                    mybir.AluOpType.mult, mybir.AluOpType.add)

    def emit_attention(g):
        for (q0, qtw) in QT:
            av = av_ps.tile([P, 512], F32)
            sm = sm_ps.tile([P, 512], F32)
            for kc in range(KC):
                # two 2-head pair tiles, pool bufs=2. All 4 S^T matmuls are
                # emitted adjacently (row positions 0/32/64/96 disjoint ->
                # the PE streams them concurrently, 512 cycles for 4 heads);
                # the two exps then run on different engines in parallel.
                pt = pt_pool.tile([P, 4, 512], FP16)
                sts = [st_ps.tile([P, 2, 512], F32, tag="st", name=f"st{p}")
                       for p in range(2)]
                for pair in range(2):
                    for hh in range(2):
                        h = 2 * pair + hh
                        mm(sts[pair][:, hh, :qtw],
                           k_g[g][32 * h:32 * (h + 1), kc * P:(kc + 1) * P],
                           q_g[g][32 * h:32 * (h + 1), q0:q0 + qtw],
                           start=True, stop=True,
                           tile_position=(32 * h, 0))
                for pair in range(2):
                    emit_exp(pt[:, 2 * pair:2 * pair + 2, :qtw],
                             sts[pair][:, :, :qtw], 2 * qtw)
                for h in range(4):
                    mm(av[32 * h:32 * (h + 1), :qtw],
                       vT_c[kc][:, 128 * g + 32 * h:128 * g + 32 * (h + 1)],
                       pt[:, h, :qtw],
                       start=(kc == 0), stop=(kc == KC - 1),
                       tile_position=(0, 32 * h), skip_group_check=True)
                for h in range(4):
                    mm(sm[32 * h:32 * (h + 1), :qtw],
                       ones_sb[:, :],
                       pt[:, h, :qtw],
                       start=(kc == 0), stop=(kc == KC - 1),
                       tile_position=(0, 32 * h), skip_group_check=True)
            rec = small.tile([P, 512], F32, tag="rec")
            nc.vector.reciprocal_approx_fast(rec[:, :qtw], sm[:, :qtw])
            nc.vector.tensor_mul(av_sb[:, g, q0:q0 + qtw], av[:, :qtw],
                                 rec[:, :qtw])

    def emit_tail():
        # queries 1024:1152 for BOTH groups in one pass: head bank h holds
        # g0 at cols 0:128, g1 at cols 128:256. Same-row-group matmuls into
        # one bank serialize on the PE (same cells), so no concurrent
        # same-bank drains.
        q0, qtw = 1024, 128
        av = av_ps.tile([P, 512], F32)
        sm = sm_ps.tile([P, 512], F32)
        for kc in range(KC):
            pt = pt_pool.tile([P, 4, 512], FP16)
            sts = [st_ps.tile([P, 2, 512], F32, tag="st", name=f"st{p}")
                   for p in range(2)]
            for pair in range(2):
                for g in range(2):
                    for hh in range(2):
                        h = 2 * pair + hh
                        mm(sts[pair][:, hh, g * qtw:(g + 1) * qtw],
                           k_g[g][32 * h:32 * (h + 1), kc * P:(kc + 1) * P],
                           q_g[g][32 * h:32 * (h + 1), q0:q0 + qtw],
                           start=(g == 0), stop=(g == 1),
                           tile_position=(32 * h, 0), skip_group_check=True)
            for pair in range(2):
                emit_exp(pt[:, 2 * pair:2 * pair + 2, :2 * qtw],
                         sts[pair][:, :, :2 * qtw], 4 * qtw)
            for g in range(2):
                for h in range(4):
                    mm(av[32 * h:32 * (h + 1), g * qtw:(g + 1) * qtw],
                       vT_c[kc][:, 128 * g + 32 * h:128 * g + 32 * (h + 1)],
                       pt[:, h, g * qtw:(g + 1) * qtw],
                       start=(kc == 0 and g == 0), stop=(kc == KC - 1 and g == 1),
                       tile_position=(0, 32 * h), skip_group_check=True)
            for g in range(2):
                for h in range(4):
                    mm(sm[32 * h:32 * (h + 1), g * qtw:(g + 1) * qtw],
                       ones_sb[:, :],
                       pt[:, h, g * qtw:(g + 1) * qtw],
                       start=(kc == 0 and g == 0), stop=(kc == KC - 1 and g == 1),
                       tile_position=(0, 32 * h), skip_group_check=True)
        rec = small.tile([P, 512], F32, tag="rec")
        nc.vector.reciprocal_approx_fast(rec[:, :2 * qtw], sm[:, :2 * qtw])
        for g in range(2):
            nc.vector.tensor_mul(av_sb[:, g, q0:q0 + qtw],
                                 av[:, g * qtw:(g + 1) * qtw],
                                 rec[:, g * qtw:(g + 1) * qtw])

    y_r = y_d.rearrange("(co p) n -> p co n", p=P)

    def emit_proj(co, nt):
        sl = slice(nt * NT3, (nt + 1) * NT3)
        ps = qkv_ps.tile([P, NT3], F32, tag="qkv")
        for ki in range(2):
            mm(ps[:], wp_sb[:, ki, co * P:(co + 1) * P],
               av_sb[:, ki, sl],
               start=(ki == 0), stop=(ki == 1))
        nc.scalar.activation(y_sb[:, co, sl], ps[:],
                             mybir.ActivationFunctionType.Identity,
                             bias=bias_sb[:, co:co + 1])
        nc.sync.dma_start(y_r[:, co, sl], y_sb[:, co, sl])

    # emission order shapes Tile's priorities: group A's inputs first so
    # the first exp lands as early as possible. proj chunks for queries
    # 0:768 only need av_sb written by the full query tiles, so they are
    # emitted before the merged tail pass and fill its engine gaps; the
    # last proj chunk (queries 768:1152) follows the tail. Output DMA is
    # per-chunk so results ship while later chunks still compute.
    # iters>1 repeats the body — used only for steady-state timing
    # measurements (db=True double-buffers the per-iteration tiles so
    # iterations overlap; db=False reuses them, serializing iterations).
    for _ in range(iters):
        # per-group activations (separate tiles => fine-grained deps)
        q_g = [acts.tile([P, NQ], FP16, name=f"q{g}", tag=f"q{g}")
               for g in range(2)]
        k_g = [acts.tile([P, N], FP16, name=f"k{g}", tag=f"k{g}")
               for g in range(2)]
        vT_c = [acts.tile([P, C], FP16, name=f"vt{mo}", tag=f"vt{mo}")
                for mo in range(KC)]
        av_sb = acts.tile([P, 2, NQ], F32R, tag="av_sb")
        y_sb = acts.tile([P, 2, NQ], F32, tag="y_sb")
        emit_qkv_group(0)
        for mo in range(KC):
            emit_vt(mo)
        emit_attention(0)
        emit_qkv_group(1)
        emit_attention(1)
        for co in range(2):
            for nt in range(2):
                emit_proj(co, nt)
        emit_tail()
        for co in range(2):
            emit_proj(co, 2)
    ctx.close()


_NC_CACHE = {}


def build_nc(iters=1, db=False):
    key = (iters, db)
    if key not in _NC_CACHE:
        nc = bacc.Bacc("TRN2", target_bir_lowering=False, debug=False,
                       num_devices=8)
        with tile.TileContext(nc) as tc:
            emit(tc, iters=iters, db=db)
        nc.compile()
        _NC_CACHE[key] = nc
    return _NC_CACHE[key]


def make_in_maps(x, w_qkv, w_proj, b_proj):
    x = np.ascontiguousarray(np.asarray(x, np.float32)).reshape(4, C, N)
    wqkvT = np.ascontiguousarray(np.asarray(w_qkv, np.float32).T)
    wprojT = np.ascontiguousarray(np.asarray(w_proj, np.float32).T)
    bprojT = np.ascontiguousarray(np.asarray(b_proj, np.float32).reshape(2, P).T)
    in_maps = []
    for core in range(8):
        b, t = divmod(core, 2)
        in_maps.append({
            "xq": np.ascontiguousarray(x[b][:, t * NQ:(t + 1) * NQ]),
            "xf": x[b],
            "wqkvT": wqkvT,
            "wprojT": wprojT,
            "bprojT": bprojT,
        })
    return in_maps


def assemble_output(results):
    y = np.empty((4, C, N), np.float32)
    for core in range(8):
        b, t = divmod(core, 2)
        y[b][:, t * NQ:(t + 1) * NQ] = results[core]["y"]
    return y.reshape(4, C, 48, 48)


def kernel(x, w_qkv, w_proj, b_proj):
    from concourse.bass_utils import run_bass_kernel_spmd
    nc = build_nc()
    in_maps = make_in_maps(x, w_qkv, w_proj, b_proj)
    res = run_bass_kernel_spmd(nc, in_maps, core_ids=list(range(8)))
    return assemble_output(res.results)
